# revision 10
# baseline (speedup 1.0000x reference)
"""DeepSeekMoE (top-2 of 8 experts + 2x shared expert) on 8 Trainium2 NeuronCores.

Strategy (hardcoded for x:[4,2048,2048], D=2048, H=1024, E=8, K=2):
  - Host: gating (logits/top-2/softmax) in float64 numpy; expert-parallel
    dispatch -- core e receives the tokens routed to expert e, gathered,
    transposed to [D, C] and padded to a common capacity C.  Shared-expert
    work is data-parallel: core c gets tokens [1024c, 1024(c+1)).
  - Device (SPMD, same program on all 8 cores): two back-to-back FFN
    pipelines computed fully transposed (outputs [D, tokens]) so that both
    layers contract over the partition dim with natural-layout weights:
      hT = gelu(w1.T @ xT + b1); yrT = w2.T @ hT        (routed, C tokens)
      hsT = gelu(sw1.T @ xsT + sb1); ysT = sw2.T @ hsT  (shared, 1024 tokens)
    b2/sb2 biases and the top-2 gate scaling are folded into the host-side
    combine (they are affine post-matmul terms).
  - Host: out[tok] += gate * (yr + b2[e]) scatter per expert; += ys + sb2.

  All matmuls run in fp8e4 (TRN e4m3, max-normal 240) with DoubleRow perf
  mode (2 k-tiles of 128 contracted per instruction; measured 71 TMAC/s vs
  40.5 for fp16).  Operands are pre-scaled by exact powers of two into the
  fp8 normal range; descale happens in the gelu activation's `scale` and in
  the host combine.  Outputs are stored as bf16 (scaled); loads are issued
  on the SP HWDGE ring and stores on the Activation HWDGE ring so store
  descriptors never head-of-line-block load prefetches.
"""
import contextlib
import os
import sys
import numpy as np

for _p in ("/root/.axon_site/_ro/trn_rl_repo", "/root/.axon_site/_ro/pypackages",
           "/opt/trn_rl_repo", "/opt/pypackages"):
    if os.path.isdir(_p) and _p not in sys.path:
        sys.path.append(_p)

import ml_dtypes
from concourse import bacc, mybir
from concourse import tile
from concourse.bass_utils import run_bass_kernel_spmd

FP8 = ml_dtypes.float8_e4m3          # TRN-compatible e4m3 (max normal 240)

# ---- problem constants (hardcoded per spec) ----
B, S, D, H, E = 4, 2048, 2048, 1024, 8
SH = 2 * H
N = B * S                    # 8192 tokens
TOPK = 2
NCORES = 8
TS = N // NCORES             # shared-expert tokens per core (1024)
ND = D // 128                # 16 contraction chunks over D
NH = H // 128                # 8 chunks over H
NSH = SH // 128              # 16 chunks over SH
TILE_N = 512                 # token-tile width (one PSUM bank of fp32)
NTS = TS // TILE_N           # shared-expert token tiles per core (2)

# power-of-two operand scales (exact in fp32; descaled exactly later)
LG_SX = 5                    # x  * 2^5   -> |.| <~ 170  (absmax ~5.3)
LG_SW1 = 18                  # w1, sw1 (+-2^-11) * 2^18 -> |.| <= 128
LG_SH = 10                   # h  * 2^10  -> |.| <~ 100  (h absmax ~0.1)
LG_SW2R = 17                 # w2  (+-2^-10) * 2^17 -> |.| <= 128
LG_SW2S = 18                 # sw2 (+-2^-11) * 2^18 -> |.| <= 128
ACT_SCALE = 2.0 ** -(LG_SX + LG_SW1)   # psum1 -> true pre-activation
YR_DESCALE = 2.0 ** -(LG_SH + LG_SW2R)
YS_DESCALE = 2.0 ** -(LG_SH + LG_SW2S)

STRATEGY = os.environ.get("MOE_STRATEGY", "f8")

F32 = mybir.dt.float32
F16 = mybir.dt.float16
BF16 = mybir.dt.bfloat16
DM8 = mybir.dt.float8e4
DR = mybir.MatmulPerfMode.DoubleRow

_DT = {"f8": (DM8, FP8)}

LAST_RESULTS = None          # BassKernelResults of the most recent device run

_BUILD_CACHE = {}


def _ntiles(total):
    """(offset, width) token tiles covering `total`.

    Default: equal-width tiles (ceil split into ceil(total/512) pieces) --
    per-instruction overhead is fixed, so equal widths beat 512s + a thin
    remainder (which runs at half MAC rate).  MOE_EQW=0 restores 512+tail.
    """
    if os.environ.get("MOE_EQW", "1") == "1" and total > TILE_N:
        k = (total + TILE_N - 1) // TILE_N
        w0 = (total + k - 1) // k
        out = []
        t = 0
        while t < total:
            w = min(w0, total - t)
            out.append((t, w))
            t += w
        return out
    out = []
    t = 0
    while t < total:
        w = min(TILE_N, total - t)
        out.append((t, w))
        t += w
    return out


def _build(C, strategy="f8", loop_iters=None):
    """Build + compile the per-core SPMD program for capacity C.

    loop_iters: if set, wrap the whole body in a device-side For_i that
    repeats it that many times (benchmarking only -- the body is idempotent).
    """
    ps_bufs = int(os.environ.get("MOE_PS_BUFS", "4"))
    no_store = os.environ.get("MOE_NO_STORE") == "1"
    fake_rhs = os.environ.get("MOE_FAKE_RHS") == "1"      # bench only
    store_eng = os.environ.get("MOE_STORE_ENG", "gp")     # gp|act|sync
    eqw = os.environ.get("MOE_EQW", "1")
    dup_r = int(os.environ.get("MOE_DUP_R", "1"))
    dup_s = int(os.environ.get("MOE_DUP_S", "1"))
    key = (C, strategy, loop_iters, ps_bufs, no_store, fake_rhs, store_eng,
           eqw, dup_r, dup_s)
    if key in _BUILD_CACHE:
        return _BUILD_CACHE[key]

    nc = bacc.Bacc("TRN2", target_bir_lowering=False, debug=False)

    # Weights: per-column-tile layout [ncols, 128(p), kchunks*128], element
    # (col, p, kc*128+c) = w[kc*128 + p, col*128 + c].
    # x: flat, blocked per token tile: block(n) is [128(p), ND*nw] with
    # element (p, d*nw+c) = xT2d[d*128 + p, n0 + c].
    # y: flat, blocked per (token tile, dt): tile block [128(p), nw] =
    # yT2d[dt*128 + p, n0:n0+nw]; for one n the ND blocks are consecutive.
    d_xT = nc.dram_tensor("xT", [D * C], DM8, kind="ExternalInput")
    d_w1 = nc.dram_tensor("w1", [NH, 128, ND * 128], DM8, kind="ExternalInput")
    d_w2 = nc.dram_tensor("w2", [ND, 128, NH * 128], DM8, kind="ExternalInput")
    d_b1 = nc.dram_tensor("b1c", [H, 1], F32, kind="ExternalInput")
    d_xsT = nc.dram_tensor("xsT", [NTS, 128, ND * TILE_N], DM8, kind="ExternalInput")
    d_sw1 = nc.dram_tensor("sw1", [NSH, 128, ND * 128], DM8, kind="ExternalInput")
    d_sw2 = nc.dram_tensor("sw2", [ND, 128, NSH * 128], DM8, kind="ExternalInput")
    d_sb1 = nc.dram_tensor("sb1c", [SH, 1], F32, kind="ExternalInput")
    d_yrT = nc.dram_tensor("yrT", [D * C], BF16, kind="ExternalOutput")
    d_ysT = nc.dram_tensor("ysT", [NTS, ND, 128, TILE_N], BF16, kind="ExternalOutput")

    v_b1 = d_b1.ap().rearrange("(b p) o -> p b o", p=128)     # [128, NH, 1]
    v_sb1 = d_sb1.ap().rearrange("(b p) o -> p b o", p=128)   # [128, NSH, 1]

    def x_tile_view(n0, nw):
        a = d_xT.ap()[D * n0: D * (n0 + nw)]
        return a.rearrange("(p d c) -> p d c", p=128, d=ND)

    def xs_tile_view(n):
        return d_xsT.ap()[n].rearrange("p (d c) -> p d c", d=ND)

    def yr_tile_view(n0, nw, dt):
        a = d_yrT.ap()[D * n0 + dt * 128 * nw: D * n0 + (dt + 1) * 128 * nw]
        return a.rearrange("(p c) -> p c", p=128)

    def wcol_view(d_w, col, kchunks):
        return d_w.ap()[col].rearrange("p (k c) -> p k c", k=kchunks)

    gelu = mybir.ActivationFunctionType.Gelu
    rtiles = _ntiles(C)

    store_ring = {"gp": "gpsimd", "act": "scalar", "sync": "sync"}[store_eng]

    with tile.TileContext(nc) as tc:
        with tc.tile_pool(name="wres", bufs=1) as wres, \
             tc.tile_pool(name="xs", bufs=4) as xs, \
             tc.tile_pool(name="hp", bufs=2) as hp, \
             tc.tile_pool(name="tmp", bufs=3) as tmpp, \
             tc.tile_pool(name="bias", bufs=1) as bias, \
             tc.tile_pool(name="ps1", bufs=ps_bufs, space="PSUM") as ps1, \
             tc.tile_pool(name="ps2", bufs=8 - ps_bufs, space="PSUM") as ps2, \
             tc.tile_pool(name="st", bufs=8) as stp:
          with (tc.For_i(0, loop_iters, 1) if loop_iters is not None
                else contextlib.nullcontext()):

            # biases first (tiny; the SP DMA FIFO runs in emission order and
            # the first gelu needs b1), then the first token tile.
            b1t = bias.tile([128, NH], F32, tag="b1")
            sb1t = bias.tile([128, NSH], F32, tag="sb1")
            nc.sync.dma_start(out=b1t[:], in_=v_b1[:, :, 0])
            nc.sync.dma_start(out=sb1t[:], in_=v_sb1[:, :, 0])

            # warm the ACT gelu table while DMAs stream: the auto-inserted
            # LoadActFuncSet binds to the first Activation in program order.
            warm = bias.tile([128, 1], F32, tag="warm")
            nc.vector.memset(warm[:], 0.0)
            nc.scalar.activation(warm[:], warm[:], gelu)

            rt = rtiles * dup_r

            # SP-ring emission order = fetch priority: x0, w1 (first L1 needs
            # them), x1, w2, x2, shared-x, then resident shared weights.
            xts = {}
            def x_dma(n):
                if n >= len(rt):
                    return
                n0, nw = rt[n]
                t = xs.tile([128, ND, TILE_N], DM8, tag="xstream")
                nc.sync.dma_start(out=t[:, :, :nw], in_=x_tile_view(n0, nw))
                xts[n] = t

            x_dma(0)
            w1cols = []
            for ht in range(NH):
                t = wres.tile([128, ND, 128], DM8, tag=f"w1c{ht}")
                nc.sync.dma_start(out=t[:], in_=wcol_view(d_w1, ht, ND))
                w1cols.append(t)
            x_dma(1)
            w2cols = []
            for dt in range(ND):
                t = wres.tile([128, NH, 128], DM8, tag=f"w2c{dt}")
                nc.sync.dma_start(out=t[:], in_=wcol_view(d_w2, dt, NH))
                w2cols.append(t)
            x_dma(2)
            xsts = []
            for n in range(NTS):
                xst = xs.tile([128, ND, TILE_N], DM8, tag="xshared")
                nc.sync.dma_start(out=xst[:], in_=xs_tile_view(n))
                xsts.append(xst)
            sw1cols = []
            for st in range(NSH):
                t = wres.tile([128, ND, 128], DM8, tag=f"sw1c{st}")
                nc.sync.dma_start(out=t[:], in_=wcol_view(d_sw1, st, ND))
                sw1cols.append(t)
            sw2cols = []
            for dt in range(ND):
                t = wres.tile([128, NSH, 128], DM8, tag=f"sw2c{dt}")
                nc.sync.dma_start(out=t[:], in_=wcol_view(d_sw2, dt, NSH))
                sw2cols.append(t)

            def store(dram_ap, psum_ap, nw):
                """psum -> bf16 staging (DVE) -> DRAM on the store ring."""
                if no_store:
                    return
                ot = stp.tile([128, TILE_N], BF16, tag="stage")
                nc.vector.tensor_copy(ot[:, :nw], psum_ap)
                getattr(nc, store_ring).dma_start(out=dram_ap, in_=ot[:, :nw])

            def act_h(h_ap, psum_ap, bias_ap, nw):
                """psum -> gelu(true scale) -> f16 tmp -> *2^LG_SH -> fp8."""
                tmp = tmpp.tile([128, TILE_N], F16, tag="tmp")
                nc.scalar.activation(tmp[:, :nw], psum_ap, gelu,
                                     bias=bias_ap, scale=ACT_SCALE)
                nc.vector.tensor_scalar_mul(h_ap, tmp[:, :nw],
                                            float(2.0 ** LG_SH))

            # ---- routed phase (software-pipelined: L1(n+1) before L2(n)
            # so L2 never waits on the gelu/mul chain of its own tile) ----
            xfix = xts[0]      # bench-only: constant rhs to break data deps

            def emit_l1(n):
                n0, nw = rt[n]
                ht_t = hp.tile([128, NH, TILE_N], DM8, tag="h")
                xt = xts.pop(n)
                if fake_rhs:
                    xt = xfix
                for ht in range(NH):
                    wv = w1cols[ht]
                    pt = ps1.tile([128, TILE_N], F32, tag="p1")
                    for d in range(ND // 2):
                        nc.tensor.matmul(pt[:, :nw], lhsT=wv[:, 2 * d:2 * d + 2, :],
                                         rhs=xt[:, 2 * d:2 * d + 2, :nw],
                                         start=(d == 0), stop=(d == ND // 2 - 1),
                                         perf_mode=DR)
                    act_h(ht_t[:, ht, :nw], pt[:, :nw], b1t[:, ht:ht + 1], nw)
                return ht_t

            def emit_l2(n, ht_t):
                n0, nw = rt[n]
                rhs_t = xfix if fake_rhs else ht_t
                for dt in range(ND):
                    wv = w2cols[dt]
                    pt2 = ps2.tile([128, TILE_N], F32, tag="p2")
                    for h in range(NH // 2):
                        nc.tensor.matmul(pt2[:, :nw], lhsT=wv[:, 2 * h:2 * h + 2, :],
                                         rhs=rhs_t[:, 2 * h:2 * h + 2, :nw],
                                         start=(h == 0), stop=(h == NH // 2 - 1),
                                         perf_mode=DR)
                    store(yr_tile_view(n0, nw, dt), pt2[:, :nw], nw)

            prev = None
            for n in range(len(rt)):
                if n >= 3:
                    x_dma(n)       # beyond the 3 prefetched at the head
                ht_t = emit_l1(n)
                if prev is not None:
                    emit_l2(n - 1, prev)
                prev = ht_t
            if prev is not None:
                emit_l2(len(rt) - 1, prev)

            # ---- shared expert phase (weights resident; st-outer) ----
            for _sdup in range(dup_s):
                hsts = []
                for _ in range(NTS):
                    hst = hp.tile([128, NSH, TILE_N], DM8, tag="hs")
                    hsts.append(hst)
                for st in range(NSH):
                    swv = sw1cols[st]
                    for n in range(NTS):
                        pt = ps1.tile([128, TILE_N], F32, tag="p1")
                        for d in range(ND // 2):
                            nc.tensor.matmul(pt[:], lhsT=swv[:, 2 * d:2 * d + 2, :],
                                             rhs=xsts[n][:, 2 * d:2 * d + 2, :],
                                             start=(d == 0), stop=(d == ND // 2 - 1),
                                             perf_mode=DR)
                        act_h(hsts[n][:, st, :], pt[:], sb1t[:, st:st + 1], TILE_N)
                for dt in range(ND):
                    swv2 = sw2cols[dt]
                    for n in range(NTS):
                        pt2 = ps2.tile([128, TILE_N], F32, tag="p2")
                        for sc in range(NSH // 2):
                            nc.tensor.matmul(pt2[:], lhsT=swv2[:, 2 * sc:2 * sc + 2, :],
                                             rhs=hsts[n][:, 2 * sc:2 * sc + 2, :],
                                             start=(sc == 0), stop=(sc == NSH // 2 - 1),
                                             perf_mode=DR)
                        store(d_ysT[n, dt, :, :], pt2[:], TILE_N)

    nc.compile()
    _BUILD_CACHE[key] = nc
    return nc


def _route(xf, gate_w):
    """float64 gating: top-2 indices (lax.top_k tie-break) + softmax gates."""
    logits = xf.astype(np.float64) @ np.asarray(gate_w).astype(np.float64)
    order = np.argsort(-logits, axis=1, kind="stable")
    idx = order[:, :TOPK]                                           # [N, 2]
    tl = np.take_along_axis(logits, idx, axis=1)
    tl = tl - tl.max(axis=1, keepdims=True)
    eg = np.exp(tl)
    gates = eg / eg.sum(axis=1, keepdims=True)                      # [N, 2]
    return idx, gates


def _q8(w, lg2):
    """Exact power-of-two scale then cast to TRN e4m3 (clip to +-240)."""
    v = np.asarray(w, np.float32) * np.float32(2.0 ** lg2)
    return np.clip(v, -240.0, 240.0).astype(FP8)


def _blockT(w, lg2):
    """[K, M] weight -> per-column-tile fp8 layout [M/128, 128(p), (K/128)*128]
    with element (col, p, kc*128 + c) = w[kc*128 + p, col*128 + c]."""
    K, M = w.shape
    r = _q8(w, lg2).reshape(K // 128, 128, M // 128, 128)
    return np.ascontiguousarray(r.transpose(2, 1, 0, 3)).reshape(M // 128, 128, K)


def _pack_x(xT2d, tiles):
    """fp8 [D, C] -> flat blocked per tile: block(n) [128, ND*nw],
    (p, d*nw+c) = xT2d[d*128+p, n0+c]."""
    r = xT2d.reshape(ND, 128, xT2d.shape[1])
    parts = [np.ascontiguousarray(r[:, :, n0:n0 + nw].transpose(1, 0, 2)).ravel()
             for n0, nw in tiles]
    return np.concatenate(parts)


def _unpack_yr(flat, C):
    """Inverse of the yrT blocked layout -> descaled [D, C] float32."""
    y = np.empty((D, C), np.float32)
    for n0, nw in _ntiles(C):
        y[:, n0:n0 + nw] = flat[D * n0: D * (n0 + nw)].astype(np.float32).reshape(D, nw)
    return y * np.float32(YR_DESCALE)


def _prepare(x, gate_w, w1, b1, w2, shared_w1, shared_b1, shared_w2, npdt=None):
    """Host routing + per-core input maps. Returns (C, in_maps, perm, gsel)."""
    xf = np.ascontiguousarray(np.asarray(x).reshape(N, D))
    idx, gates = _route(xf, gate_w)

    perm = []      # token ids routed to each expert (ascending)
    gsel = []      # matching gate weight
    for e in range(E):
        hit0 = idx[:, 0] == e
        hit1 = idx[:, 1] == e
        sel = np.where(hit0 | hit1)[0]
        g = np.where(hit0[sel], gates[sel, 0], gates[sel, 1])
        perm.append(sel)
        gsel.append(g)
    cmax = max(len(p) for p in perm)
    C = ((cmax + 127) // 128) * 128
    rtiles = _ntiles(C)
    stiles = _ntiles(TS)

    xfq = _q8(xf, LG_SX)                                   # [N, D] fp8
    sw1b = _blockT(shared_w1, LG_SW1)
    sw2b = _blockT(shared_w2, LG_SW2S)
    sb1c = np.ascontiguousarray(np.asarray(shared_b1).astype(np.float32)).reshape(SH, 1)
    in_maps = []
    for c in range(E):
        sel = perm[c]
        xT = np.zeros((D, C), FP8)
        xT[:, :len(sel)] = xfq[sel].T
        xsT = np.ascontiguousarray(xfq[c * TS:(c + 1) * TS].T)
        in_maps.append({
            "xT": _pack_x(xT, rtiles),
            "w1": _blockT(w1[c], LG_SW1),
            "w2": _blockT(w2[c], LG_SW2R),
            "b1c": np.ascontiguousarray(np.asarray(b1[c]).astype(np.float32)).reshape(H, 1),
            "xsT": _pack_x(xsT, stiles).reshape(NTS, 128, ND * TILE_N),
            "sw1": sw1b,
            "sw2": sw2b,
            "sb1c": sb1c,
        })
    return C, in_maps, perm, gsel


def kernel(x, gate_w, w1, b1, w2, b2, shared_w1, shared_b1, shared_w2, shared_b2):
    global LAST_RESULTS
    C, in_maps, perm, gsel = _prepare(
        x, gate_w, w1, b1, w2, shared_w1, shared_b1, shared_w2)
    nc = _build(C, STRATEGY)

    LAST_RESULTS = run_bass_kernel_spmd(nc, in_maps, core_ids=list(range(NCORES)))
    res = LAST_RESULTS.results

    b2 = np.asarray(b2)
    shared_b2 = np.asarray(shared_b2)
    out = np.zeros((N, D), np.float64)
    for c in range(E):
        sel = perm[c]
        yr = _unpack_yr(res[c]["yrT"], C).T[:len(sel)].astype(np.float64)
        out[sel] += gsel[c][:, None] * (yr + b2[c].astype(np.float64))
        ys = res[c]["ysT"].astype(np.float32).reshape(NTS, D, TILE_N)
        ys2d = np.concatenate([ys[n] for n in range(NTS)], axis=1)  # [D, TS]
        out[c * TS:(c + 1) * TS] += (ys2d.T.astype(np.float64) * YS_DESCALE
                                     + shared_b2.astype(np.float64))

    return out.reshape(B, S, D).astype(np.float32)
